# revision 13
# baseline (speedup 1.0000x reference)
"""Trainium2 Bass kernel for nn_CARRVProj (moe_routing).

Math (per token row v of V = x @ Wv.T + bv):
  r  = v @ Wg.T + bg                      router logits            (E)
  pv = Wp[e,p,:] . v                      probe projections        (E,P)
  c  = ||pv||_2 / sqrt(P)                 capability scores        (E)
  s  = LN_E(r)*g_r+b_r + sigmoid(alpha)*(LN_E(c)*g_c+b_c)
  top-2(s) -> softmax -> per-expert weights we (0 for unselected)
  y  = v + sum_e we[e] * (silu(v @ W1[e].T + b1[e]) @ W2[e].T + b2[e])

Strategy: data-parallel over the 16384 flattened tokens across 8 cores
(2048 tokens/core), weights replicated.

Precision plan: the V projection / expert path runs in BF16 (1 PE
cycle/row, half the HBM and SBUF traffic of fp32; the ~4e-3 absolute
error is far inside the 2e-2 gate). The router scores feed a top-2
selection where rounding can flip expert choices, so the router path
runs in TRUE fp32 from a separate fp32 copy of x: the router weights
are composed with the value projection on the host
(Wxrt = Wv.T @ [Wg;Wp].T in float64), and x-stationary fp32 matmuls
([128-token stationary] x [72 moving router columns]) produce
token-major scores directly — 72x4 streamed cycles per
(h-chunk, token-chunk) instead of 512x4, and no score transpose.

Per 512-token block: V.T via bf16; token-major router scores via fp32;
LN + top-2 via DVE max8; softmax-of-2 == sigmoid(v1-v2); routing
weights spread to the stacked-expert axis with a 0/1 selection matmul;
the expert down-proj is host-composed with Wv (W1c = W1 @ Wv) so it
contracts over H straight from x; silu as h*sigmoid(h); up-proj runs
DV-major (out = delta.T) accumulating on top of an identity-matmul
copy of V.T so the "+v" residual add is free and no PE transposes are
needed; y is stored DV-major in bf16 and transposed/widened on the
host. DMA queues: fp32 x on SP, bf16 x on ACT, y stores on gpsimd;
PSUM->SBUF output copies run on DVE to keep ACT off the critical path.
"""

import numpy as np

# ---------------------------------------------------------------- problem dims
B, S, H, DV = 4, 4096, 1024, 1024
E, INNER, PPROBE, TOPK = 8, 32, 8, 2
N_CORES = 8
NTOK = B * S                 # 16384 flattened tokens
NC_TOK = NTOK // N_CORES     # 2048 tokens per core
TBLK = 512                   # tokens per block
NBLK = NC_TOK // TBLK        # 4 blocks per core
NRT = E + E * PPROBE         # 72 fused router rows (8 logits + 64 probes)
EI = E * INNER               # 256 stacked expert inner rows

_BUILD_CACHE: dict = {}


def _build(has_bv: bool, has_brt: bool, has_b1: bool, has_b2: bool,
           repeat: int = 1, max_unroll: int = 1):
    """Build + compile the SPMD single-core program (same NEFF on all cores)."""
    import concourse.bass as bass
    import concourse.tile as tile
    import concourse.mybir as mybir
    from concourse import bacc
    from contextlib import ExitStack

    ts = bass.ts
    ds = bass.ds
    f32 = mybir.dt.float32
    bf16 = mybir.dt.bfloat16
    AF = mybir.ActivationFunctionType
    OP = mybir.AluOpType
    AX = mybir.AxisListType

    nc = bacc.Bacc("TRN2", target_bir_lowering=False, debug=False,
                   num_devices=N_CORES)

    # ------------------------------------------------------------ DRAM params
    xT_d = nc.dram_tensor("xT", [H, NC_TOK], f32, kind="ExternalInput").ap()
    xT16_d = nc.dram_tensor("xT16", [H, NC_TOK], bf16,
                            kind="ExternalInput").ap()
    WvT_d = nc.dram_tensor("WvT", [H, DV], bf16, kind="ExternalInput").ap()
    # packed [p, c*r] host layouts -> single contiguous descriptor/partition
    Wxrt_d = nc.dram_tensor("Wxrt", [128, 8 * NRT], f32,
                            kind="ExternalInput").ap()
    W1T_d = nc.dram_tensor("W1T", [128, 8 * EI], bf16,
                           kind="ExternalInput").ap()
    W2c_d = nc.dram_tensor("W2c", [EI, DV], bf16, kind="ExternalInput").ap()
    b2_d = nc.dram_tensor("b2", [E, DV], bf16, kind="ExternalInput").ap()
    bv_d = nc.dram_tensor("bv", [DV], f32, kind="ExternalInput").ap()
    brt_d = nc.dram_tensor("brt", [128, NRT], f32, kind="ExternalInput").ap()
    b1_d = nc.dram_tensor("b1r", [1, EI], bf16, kind="ExternalInput").ap()
    grt_d = nc.dram_tensor("grt", [128, E], f32, kind="ExternalInput").ap()
    gct_d = nc.dram_tensor("gct", [128, E], f32, kind="ExternalInput").ap()
    bal_d = nc.dram_tensor("bal", [128, E], f32, kind="ExternalInput").ap()
    iden_d = nc.dram_tensor("iden", [128, 128], f32, kind="ExternalInput").ap()
    iden16_d = nc.dram_tensor("iden16", [128, 128], bf16,
                              kind="ExternalInput").ap()
    bsel_d = nc.dram_tensor("bsel", [E, EI], bf16, kind="ExternalInput").ap()
    ones_d = nc.dram_tensor("ones", [1, TBLK], bf16, kind="ExternalInput").ap()
    yT_d = nc.dram_tensor("yT", [DV, NC_TOK], bf16, kind="ExternalOutput").ap()

    # DRAM views of x with the h-chunk axis explicit: (c p) t -> p c t
    xT_c = xT_d.rearrange("(c p) t -> p c t", p=128)
    xT16_c = xT16_d.rearrange("(c p) t -> p c t", p=128)

    with tile.TileContext(nc) as tc, ExitStack() as ctx:
        wpool = ctx.enter_context(tc.tile_pool(name="weights", bufs=1))
        xpool = ctx.enter_context(tc.tile_pool(name="xin", bufs=2))
        vpool = ctx.enter_context(tc.tile_pool(name="vt", bufs=2))
        rpool = ctx.enter_context(tc.tile_pool(name="router", bufs=2))
        hpool = ctx.enter_context(tc.tile_pool(name="hs", bufs=2))
        ypool = ctx.enter_context(tc.tile_pool(name="yout", bufs=3))
        ps_v = ctx.enter_context(tc.tile_pool(name="ps_v", bufs=2, space="PSUM"))
        ps_m = ctx.enter_context(tc.tile_pool(name="ps_m", bufs=2, space="PSUM"))
        ps_h = ctx.enter_context(tc.tile_pool(name="ps_h", bufs=2, space="PSUM"))
        ps_y = ctx.enter_context(tc.tile_pool(name="ps_y", bufs=2, space="PSUM"))

        # -------------------- early weights: router path only (small, fast)
        Wxrt = wpool.tile([128, 8, NRT], f32)
        nc.sync.dma_start(Wxrt[:].rearrange("p c r -> p (c r)"), Wxrt_d[:])
        brt_sb = wpool.tile([128, NRT], f32)
        if has_brt:
            nc.sync.dma_start(brt_sb[:], brt_d[:])
        idf = wpool.tile([128, 128], f32)
        nc.sync.dma_start(idf[:], iden_d[:])
        epsc = wpool.tile([128, 1], f32)
        nc.vector.memset(epsc[:], 1e-5)
        grt = wpool.tile([128, E], f32)
        nc.sync.dma_start(grt[:], grt_d[:])
        gct = wpool.tile([128, E], f32)
        nc.sync.dma_start(gct[:], gct_d[:])
        bal = wpool.tile([128, E], f32)
        nc.sync.dma_start(bal[:], bal_d[:])

        def emit_x_router(b):
            """x loads + token-major router scores for block b."""
            tok0 = b * TBLK
            # fp32 x for the router (SP queue), bf16 x for V/experts (ACT)
            xf = xpool.tile([128, 8, TBLK], f32, tag="xf")
            nc.sync.dma_start(xf[:, 0:4, :], xT_c[:, 0:4, ds(tok0, TBLK)])
            nc.sync.dma_start(xf[:, 4:8, :], xT_c[:, 4:8, ds(tok0, TBLK)])
            x16 = xpool.tile([128, 8, TBLK], bf16, tag="x16")
            nc.scalar.dma_start(x16[:], xT16_c[:, :, ds(tok0, TBLK)])

            # router scores in TRUE fp32: W-stationary [72, TBLK] then
            # PE-transpose to token-major [128, 4, 72]
            rt_ps = ps_m.tile([NRT, TBLK], f32, tag="m")
            for hc in range(8):
                nc.tensor.matmul(rt_ps[:], Wxrt[:, hc, :], xf[:, hc, :],
                                 start=(hc == 0), stop=(hc == 7))
            rt = rpool.tile([NRT, TBLK], f32, tag="rt")
            if has_brt:
                nc.scalar.activation(rt[:], rt_ps[:], AF.Identity,
                                     bias=brt_sb[0:NRT, 0:1], scale=1.0)
            else:
                nc.scalar.copy(rt[:], rt_ps[:])
            rta = rpool.tile([128, 4, NRT], f32)
            for tc4 in range(4):
                rtT = ps_m.tile([128, NRT], f32, tag="m")
                nc.tensor.matmul(rtT[:], rt[:, ts(tc4, 128)],
                                 idf[:NRT, :NRT], is_transpose=True,
                                 start=True, stop=True)
                nc.scalar.copy(rta[:, tc4, :], rtT[:])
            return tok0, x16, rta

        def emit_main_weights():
            WvT = wpool.tile([128, 8, DV], bf16)
            for hc in range(8):
                eng = nc.sync if hc % 2 else nc.scalar
                eng.dma_start(WvT[:, hc, :], WvT_d[ds(hc * 128, 128), :])
            W1T = wpool.tile([128, 8, EI], bf16)
            nc.sync.dma_start(W1T[:].rearrange("p c r -> p (c r)"), W1T_d[:])
            W2c = wpool.tile([128, 2, DV], bf16)
            for kc in range(2):
                nc.sync.dma_start(W2c[:, kc, :], W2c_d[ds(kc * 128, 128), :])
            idr = wpool.tile([128, 128], bf16)
            nc.sync.dma_start(idr[:], iden16_d[:])
            bsel = wpool.tile([E, EI], bf16)
            nc.sync.dma_start(bsel[:], bsel_d[:])
            ones = wpool.tile([1, TBLK], bf16)
            nc.sync.dma_start(ones[:], ones_d[:])
            b2sb = wpool.tile([E, DV], bf16)
            if has_b2:
                nc.sync.dma_start(b2sb[:], b2_d[:])
            bvc = wpool.tile([128, 8], f32)
            if has_bv:
                nc.sync.dma_start(bvc[:], bv_d.rearrange("(c p) -> p c", p=128))
            b1r = wpool.tile([1, EI], bf16)
            if has_b1:
                nc.sync.dma_start(b1r[:], b1_d[:])
            return WvT, W1T, W2c, idr, bsel, ones, b2sb, bvc, b1r

        def emit_block_rest(hdl, weights):
            tok0, x16, rta = hdl
            WvT, W1T, W2c, idr, bsel, ones, b2sb, bvc, b1r = weights

            # ------------------------------- V.T = Wv @ x.T  (bf16)  +bv
            VT = vpool.tile([128, 8, TBLK], bf16)
            for dvc in range(8):
                pv = ps_v.tile([128, TBLK], f32)
                for hc in range(8):
                    nc.tensor.matmul(pv[:], WvT[:, hc, ts(dvc, 128)],
                                     x16[:, hc, :],
                                     start=(hc == 0), stop=(hc == 7))
                if has_bv:
                    nc.scalar.activation(VT[:, dvc, :], pv[:], AF.Identity,
                                         bias=bvc[:, ds(dvc, 1)], scale=1.0)
                else:
                    nc.scalar.copy(VT[:, dvc, :], pv[:])

            # ------------------------------------------- router math (DVE/ACT)
            r_v = rta[:, :, 0:E]                       # [128,4,8]
            pv_v = rta[:, :, E:NRT]                    # [128,4,64]
            sc = rpool.tile([128, 4, 24], f32)         # ctr_r | ctr_c | s
            ctr_r = sc[:, :, 0:8]
            ctr_c = sc[:, :, 8:16]
            s_all = sc[:, :, 16:24]
            st = rpool.tile([128, 4, 8], f32)          # mr|mc|vr|vc|d|a|a1m|--
            pvsq = rpool.tile([128, 4, 64], f32)
            c_t = rpool.tile([128, 4, 8], f32)
            vmax = rpool.tile([128, 4, 8], f32)
            we = rpool.tile([128, 4, 8], f32)

            nc.scalar.square(pvsq[:], pv_v)
            nc.vector.reduce_sum(
                out=c_t[:], in_=pvsq[:].rearrange("p c (e q) -> p c e q", q=8),
                axis=AX.X)
            # c = sqrt(sum_p pv^2 / P)  (keep exact scale: eps inside LN)
            nc.scalar.activation(c_t[:], c_t[:], AF.Sqrt, bias=0.0,
                                 scale=1.0 / PPROBE)
            # LN statistics over the expert axis (free dim of size 8)
            nc.vector.tensor_reduce(out=st[:, :, 0:1], in_=r_v, axis=AX.X,
                                    op=OP.add)
            nc.vector.tensor_reduce(out=st[:, :, 1:2], in_=c_t[:], axis=AX.X,
                                    op=OP.add)
            nc.vector.tensor_scalar(st[:, :, 0:2], st[:, :, 0:2], 1.0 / E,
                                    None, OP.mult)
            nc.vector.tensor_tensor(ctr_r, r_v,
                                    st[:, :, 0:1].broadcast_to([128, 4, 8]),
                                    OP.subtract)
            nc.vector.tensor_tensor(ctr_c, c_t[:],
                                    st[:, :, 1:2].broadcast_to([128, 4, 8]),
                                    OP.subtract)
            nc.vector.tensor_tensor(pvsq[:, :, 0:8], ctr_r, ctr_r, OP.mult)
            nc.vector.tensor_tensor(pvsq[:, :, 8:16], ctr_c, ctr_c, OP.mult)
            nc.vector.reduce_sum(
                out=st[:, :, 2:4],
                in_=pvsq[:, :, 0:16].rearrange("p c (e q) -> p c e q", q=8),
                axis=AX.X)
            # std = sqrt(var + eps) ; istd = 1/std
            nc.scalar.activation(st[:, :, 2:4], st[:, :, 2:4], AF.Sqrt,
                                 bias=epsc[:, 0:1], scale=1.0 / E)
            nc.vector.reciprocal(st[:, :, 2:4], st[:, :, 2:4])
            # s = LN(r)*g_r + LN(c)*(sig(alpha)*g_c) + (b_r + sig(alpha)*b_c)
            nc.vector.tensor_tensor(ctr_r, ctr_r,
                                    st[:, :, 2:3].broadcast_to([128, 4, 8]),
                                    OP.mult)
            nc.vector.tensor_tensor(ctr_c, ctr_c,
                                    st[:, :, 3:4].broadcast_to([128, 4, 8]),
                                    OP.mult)
            nc.vector.tensor_tensor(
                ctr_r, ctr_r,
                grt[:].unsqueeze(1).broadcast_to([128, 4, 8]), OP.mult)
            nc.vector.tensor_tensor(
                ctr_c, ctr_c,
                gct[:].unsqueeze(1).broadcast_to([128, 4, 8]), OP.mult)
            nc.vector.tensor_tensor(s_all, ctr_r, ctr_c, OP.add)
            nc.vector.tensor_tensor(
                s_all, s_all,
                bal[:].unsqueeze(1).broadcast_to([128, 4, 8]), OP.add)
            # top-2 + softmax-of-2 (sigmoid of score gap)
            for c4 in range(4):
                nc.vector.max(out=vmax[:, c4, :], in_=s_all[:, c4, :])
            nc.vector.tensor_tensor(st[:, :, 4:5], vmax[:, :, 0:1],
                                    vmax[:, :, 1:2], OP.subtract)
            nc.scalar.activation(st[:, :, 5:6], st[:, :, 4:5], AF.Sigmoid,
                                 bias=0.0, scale=1.0)
            nc.vector.tensor_scalar(st[:, :, 6:7], st[:, :, 5:6], -1.0, 1.0,
                                    OP.mult, OP.add)  # 1 - a
            # we = (s==v1)*a + (s==v2)*(1-a)
            nc.vector.tensor_tensor(we[:], s_all,
                                    vmax[:, :, 0:1].broadcast_to([128, 4, 8]),
                                    OP.is_equal)
            nc.vector.tensor_tensor(we[:], we[:],
                                    st[:, :, 5:6].broadcast_to([128, 4, 8]),
                                    OP.mult)
            nc.vector.tensor_tensor(pvsq[:, :, 16:24], s_all,
                                    vmax[:, :, 1:2].broadcast_to([128, 4, 8]),
                                    OP.is_equal)
            nc.vector.tensor_tensor(pvsq[:, :, 16:24], pvsq[:, :, 16:24],
                                    st[:, :, 6:7].broadcast_to([128, 4, 8]),
                                    OP.mult)
            nc.vector.tensor_tensor(we[:], we[:], pvsq[:, :, 16:24], OP.add)

            # ------------------------- weT [8, TBLK] (transpose back, fp32)
            weT_ps = ps_m.tile([E, TBLK], f32, tag="m")
            for c4 in range(4):
                nc.tensor.matmul(weT_ps[:, ts(c4, 128)], we[:, c4, :],
                                 idf[:], is_transpose=True,
                                 start=(c4 == 0), stop=(c4 == 3))
            weT = rpool.tile([E, TBLK], bf16)
            nc.scalar.copy(weT[:], weT_ps[:])

            # ---------------- experts down-proj + silu + routing-weight scale
            # W1T is host-composed with Wv (W1c = W1 @ Wv), so the down-proj
            # contracts over H straight from x — no dependency on VT.
            hs = hpool.tile([128, 2, TBLK], bf16)
            for g2 in range(2):
                h_ps = ps_h.tile([128, TBLK], f32)
                for hc in range(8):
                    nc.tensor.matmul(h_ps[:], W1T[:, hc, ts(g2, 128)],
                                     x16[:, hc, :],
                                     start=(hc == 0),
                                     stop=(hc == 7 and not has_b1))
                if has_b1:
                    nc.tensor.matmul(h_ps[:], b1r[:, ts(g2, 128)],
                                     ones[:], start=False, stop=True)
                wb_ps = ps_m.tile([128, TBLK], f32, tag="m")
                nc.tensor.matmul(wb_ps[:], bsel[:, ts(g2, 128)],
                                 weT[:], start=True, stop=True)
                sg = hpool.tile([128, TBLK], f32, tag="sg")
                nc.scalar.activation(sg[:], h_ps[:], AF.Sigmoid, bias=0.0,
                                     scale=1.0)
                nc.vector.tensor_tensor(sg[:], sg[:], h_ps[:], OP.mult)
                nc.vector.tensor_tensor(hs[:, g2, :], sg[:], wb_ps[:], OP.mult)

            # ----- y.T = V.T + delta.T per 128-dv chunk, residual via identity
            for dvc in range(8):
                y_ps = ps_y.tile([128, TBLK], f32)
                nc.tensor.matmul(y_ps[:], idr[:], VT[:, dvc, :],
                                 start=True, stop=False)
                for g2 in range(2):
                    nc.tensor.matmul(
                        y_ps[:], W2c[:, g2, ts(dvc, 128)], hs[:, g2, :],
                        start=False,
                        stop=(g2 == 1 and not has_b2))
                if has_b2:
                    nc.tensor.matmul(y_ps[:], b2sb[:, ts(dvc, 128)],
                                     weT[:], start=False, stop=True)
                ysb = ypool.tile([128, TBLK], bf16)
                nc.vector.tensor_scalar(ysb[:], y_ps[:], 1.0, None, OP.mult)
                (nc.sync if dvc % 2 else nc.gpsimd).dma_start(
                    yT_d[ds(dvc * 128, 128), ds(tok0, TBLK)], ysb[:])

        if repeat == 1:
            h0 = emit_x_router(0)
            weights = emit_main_weights()
            emit_block_rest(h0, weights)
            for b in range(1, NBLK):
                emit_block_rest(emit_x_router(b), weights)
        else:
            weights = emit_main_weights()

            def body(_i):
                for b in range(NBLK):
                    emit_block_rest(emit_x_router(b), weights)
            tc.For_i_unrolled(0, repeat, 1, body, max_unroll=max_unroll)

    nc.compile()
    return nc


def _get_built(key):
    if key not in _BUILD_CACHE:
        _BUILD_CACHE[key] = _build(*key)
    return _BUILD_CACHE[key]


def _host_prep(x, Wv, bv, Wg, bg, Wp, alpha, g_r, b_r, g_c, b_c,
               W1, b1, W2, b2):
    import ml_dtypes
    f = np.float32
    bf = ml_dtypes.bfloat16
    xf = np.ascontiguousarray(np.asarray(x, f).reshape(NTOK, H))
    xT = np.ascontiguousarray(xf.T)                                # [H, NTOK]
    xT16 = np.ascontiguousarray(xT.astype(bf))
    WvT = np.ascontiguousarray(np.asarray(Wv, f).T.astype(bf))     # [H, DV]
    Wrt = np.concatenate([np.asarray(Wg, f),
                          np.asarray(Wp, f).reshape(E * PPROBE, DV)], 0)
    # Compose router with the value projection in float64:
    #   rt = V @ Wrt.T = x @ (Wv.T @ Wrt.T) + (Wrt @ bv)
    Wxrt_hr = (np.asarray(Wv, np.float64).T
               @ np.asarray(Wrt, np.float64).T).astype(f)          # [H, 72]
    # pack [c*128+p, r] -> [p, c*72+r] so the DMA is contiguous per partition
    Wxrt = np.ascontiguousarray(
        Wxrt_hr.reshape(8, 128, NRT).transpose(1, 0, 2).reshape(128, 8 * NRT))
    brt = (np.asarray(Wrt, np.float64) @ np.asarray(bv, np.float64)).astype(f)
    brt[:E] += np.asarray(bg, f)
    brt_bc = np.ascontiguousarray(
        np.broadcast_to(brt.reshape(1, NRT), (128, NRT)))
    # compose the down-proj with the value projection (float64 on host):
    #   h = V @ W1.T + b1 = x @ (Wv.T @ W1.T) + (W1 @ bv + b1)
    W1f64 = np.asarray(W1, np.float64).reshape(EI, DV)
    W1c = (W1f64 @ np.asarray(Wv, np.float64)).astype(f)          # [EI, H]
    W1T = np.ascontiguousarray(
        W1c.T                                          # [H, 256]
        .reshape(8, 128, EI).transpose(1, 0, 2).reshape(128, 8 * EI)
        .astype(bf))
    b1c = (W1f64 @ np.asarray(bv, np.float64)
           + np.asarray(b1, np.float64).reshape(EI)).astype(f)
    W2c = np.ascontiguousarray(
        np.transpose(np.asarray(W2, f), (0, 2, 1)).reshape(EI, DV).astype(bf))
    sig = float(1.0 / (1.0 + np.exp(-np.float64(np.asarray(alpha)))))
    grt = np.ascontiguousarray(
        np.broadcast_to(np.asarray(g_r, f).reshape(1, E), (128, E)))
    gct = np.ascontiguousarray(
        np.broadcast_to((sig * np.asarray(g_c, f)).reshape(1, E), (128, E)))
    bal = np.ascontiguousarray(np.broadcast_to(
        (np.asarray(b_r, f) + sig * np.asarray(b_c, f)).reshape(1, E),
        (128, E)))
    b1r = np.ascontiguousarray(b1c.reshape(1, EI).astype(bf))
    iden = np.eye(128, dtype=f)
    bsel = np.zeros((E, EI), f)
    for e in range(E):
        bsel[e, e * INNER:(e + 1) * INNER] = 1.0
    ones = np.ones((1, TBLK), bf)
    common = {
        "WvT": WvT, "Wxrt": Wxrt, "W1T": W1T, "W2c": W2c,
        "b2": np.ascontiguousarray(np.asarray(b2, f).astype(bf)),
        "bv": np.ascontiguousarray(np.asarray(bv, f)),
        "brt": brt_bc, "b1r": b1r, "grt": grt, "gct": gct, "bal": bal,
        "iden": iden, "iden16": np.ascontiguousarray(iden.astype(bf)),
        "bsel": np.ascontiguousarray(bsel.astype(bf)), "ones": ones,
    }
    flags = (bool(np.any(np.asarray(bv))), bool(np.any(brt)),
             bool(np.any(b1c)), bool(np.any(np.asarray(b2))))
    in_maps = []
    for c in range(N_CORES):
        m = dict(common)
        m["xT"] = np.ascontiguousarray(xT[:, c * NC_TOK:(c + 1) * NC_TOK])
        m["xT16"] = np.ascontiguousarray(xT16[:, c * NC_TOK:(c + 1) * NC_TOK])
        in_maps.append(m)
    return in_maps, flags


def kernel(x, Wv, bv, Wg, bg, Wp, alpha, g_r, b_r, g_c, b_c, W1, b1, W2, b2):
    from concourse.bass_utils import run_bass_kernel_spmd
    in_maps, flags = _host_prep(x, Wv, bv, Wg, bg, Wp, alpha,
                                g_r, b_r, g_c, b_c, W1, b1, W2, b2)
    nc = _get_built((*flags, 1, 1))
    res = run_bass_kernel_spmd(nc, in_maps, core_ids=list(range(N_CORES)))
    y = np.concatenate(
        [np.asarray(res.results[c]["yT"]).astype(np.float32).T
         for c in range(N_CORES)], 0)
    return y.reshape(B, S, DV)
